# revision 6
# baseline (speedup 1.0000x reference)
"""Trainium2 Bass kernel for nn_MixedSparseGatedMLP (LoRA-augmented gated MLP).

Math (reference):
    y1 = x @ Wg + (x @ Ag) @ Bg
    y2 = x @ Wu + (x @ Au) @ Bu
    x3 = relu(y1) * y2
    y3 = x3 @ Wd + (x3 @ Ad) @ Bd

Strategy:
  - Fold the rank-16 LoRA factors into the dense weights on the host
    (exact fp32 algebra): Wg_eff = Wg + Ag@Bg, etc.  The device kernel is
    then a plain gated MLP with three dense matmuls.
  - Tensor-parallel over the intermediate dim I across 8 NeuronCores:
    each core owns I/8 columns of Wg_eff/Wu_eff and I/8 rows of Wd_eff,
    processes ALL tokens, and produces a partial [NTOK, H] output.
    Partials are summed on the host (gather/unshard step).
  - bf16 operands, fp32 PSUM accumulation, fp32 partial outputs.
  - All DRAM layouts are pre-tiled on the host so every DMA is a linear
    (or near-linear) copy into the exact SBUF layout the matmuls need.
"""

import os
import sys

for _p in ("/opt/trn_rl_repo", "/root/.axon_site/_ro/trn_rl_repo"):
    if os.path.isdir(_p) and _p not in sys.path:
        sys.path.append(_p)

import numpy as np
import ml_dtypes

# Problem shapes (hardcoded per contract)
B, S, H, I, R = 2, 2048, 4096, 11008, 16
NTOK = B * S              # 4096 tokens
NCORES = 8
IPAD = 11264              # I padded to 88*128 so it splits 8 ways into 128-chunks
IS = IPAD // NCORES       # 1408 intermediate columns per core
C = IS // 128             # 11 i-chunks per core
K = H // 128              # 32 h-chunks
TB = 512                  # token block
NB = NTOK // TB           # 8 token blocks
MT = TB // 128            # 4 token m-tiles per block
NH = H // 512             # 8 output n-tiles

BF16 = ml_dtypes.bfloat16

# set by test.py for profiling; harness path leaves these as-is
TRACE = False
LAST_EXEC_TIME_NS = None
LAST_RESULTS = None


def _build_nc():
    import concourse.bacc as bacc
    import concourse.mybir as mybir
    import concourse.tile as tile

    bf16 = mybir.dt.bfloat16
    f32 = mybir.dt.float32

    nc = bacc.Bacc("TRN2", target_bir_lowering=False, debug=False)

    # DRAM parameters (host pre-tiled layouts; see kernel() for the math)
    x = nc.declare_dram_parameter("x", [NB, 128, K * TB], bf16, isOutput=False)
    wg = nc.declare_dram_parameter("wg", [C, 128, K * 128], bf16, isOutput=False)
    wu = nc.declare_dram_parameter("wu", [C, 128, K * 128], bf16, isOutput=False)
    wd = nc.declare_dram_parameter("wd", [C, 128, H], bf16, isOutput=False)
    out = nc.declare_dram_parameter("out", [NTOK, H], f32, isOutput=True)

    with tile.TileContext(nc) as tc:
        with tc.tile_pool(name="xp", bufs=1) as xp, \
             tc.tile_pool(name="wp", bufs=16) as wp, \
             tc.tile_pool(name="wdp", bufs=1) as wdp, \
             tc.tile_pool(name="x3p", bufs=2) as x3p, \
             tc.tile_pool(name="rp", bufs=2) as rp, \
             tc.tile_pool(name="op", bufs=4) as op, \
             tc.tile_pool(name="pgp", bufs=2, space="PSUM") as pgp, \
             tc.tile_pool(name="pup", bufs=2, space="PSUM") as pup, \
             tc.tile_pool(name="pdp", bufs=2, space="PSUM") as pdp:

            wdt = []
            KG = 4            # k-groups per block (split DMAs so the first
            KS = K // KG      # matmuls gate on ~1MB, not the full 4MB)

            for b in range(NB):
                # x block, split into KG tiles: [128 h-in-chunk, (k, t)] bf16
                xbg = []
                for gi in range(KG):
                    t = xp.tile([128, KS * TB], bf16, tag=f"xb{gi}")
                    nc.sync.dma_start(t, x[b][:, gi * KS * TB:(gi + 1) * KS * TB])
                    xbg.append(t)

                if b == 0:
                    # Wd_eff stays SBUF-resident for the whole kernel
                    # (11 x 1MB).  Preload on the software-DGE path so it
                    # doesn't delay the block-0 x/weight DMAs on HWDGE
                    # that gate the first matmul.
                    for c in range(C):
                        t = wdp.tile([128, H], bf16, tag=f"wd{c}")
                        nc.gpsimd.dma_start(t, wd[c])
                        wdt.append(t)

                # x3^T for this block: [128 i-in-chunk, (c, t)] bf16
                x3 = x3p.tile([128, C * TB], bf16, tag="x3")

                # ---- gate / up projections + gating, per i-chunk m ----
                for m in range(C):
                    wgt = []
                    wut = []
                    for gi in range(KG):
                        t = wp.tile([128, KS * 128], bf16, tag="w")
                        nc.sync.dma_start(t, wg[m][:, gi * KS * 128:(gi + 1) * KS * 128])
                        wgt.append(t)
                    for gi in range(KG):
                        t = wp.tile([128, KS * 128], bf16, tag="w")
                        nc.sync.dma_start(t, wu[m][:, gi * KS * 128:(gi + 1) * KS * 128])
                        wut.append(t)

                    g = pgp.tile([128, TB], f32, tag="pg")
                    u = pup.tile([128, TB], f32, tag="pu")
                    for k in range(K):
                        gi, kk = divmod(k, KS)
                        nc.tensor.matmul(
                            g,
                            wgt[gi][:, kk * 128:(kk + 1) * 128],
                            xbg[gi][:, kk * TB:(kk + 1) * TB],
                            start=(k == 0), stop=(k == K - 1),
                        )
                    for k in range(K):
                        gi, kk = divmod(k, KS)
                        nc.tensor.matmul(
                            u,
                            wut[gi][:, kk * 128:(kk + 1) * 128],
                            xbg[gi][:, kk * TB:(kk + 1) * TB],
                            start=(k == 0), stop=(k == K - 1),
                        )
                    # x3 = relu(g) * u ; DVE may read only one PSUM input,
                    # so relu lands in SBUF via ACT first.
                    r = rp.tile([128, TB], bf16, tag="r")
                    nc.scalar.activation(r, g, mybir.ActivationFunctionType.Relu)
                    nc.vector.tensor_mul(x3[:, m * TB:(m + 1) * TB], r, u)

                # ---- down projection: out[tok, h] partial ----
                for mt in range(MT):
                    for n in range(NH):
                        d = pdp.tile([128, 512], f32, tag="pd")
                        for c in range(C):
                            nc.tensor.matmul(
                                d,
                                x3[:, c * TB + mt * 128: c * TB + (mt + 1) * 128],
                                wdt[c][:, n * 512:(n + 1) * 512],
                                start=(c == 0), stop=(c == C - 1),
                            )
                        o = op.tile([128, 512], f32, tag="o")
                        nc.scalar.copy(o, d)
                        row = b * TB + mt * 128
                        # store via SWDGE: keeps HWDGE free for the
                        # x/weight prefetches that gate the next block
                        nc.gpsimd.dma_start(
                            out[row:row + 128, n * 512:(n + 1) * 512], o
                        )

    nc.compile()
    return nc


def _prep_inputs(x1, w_gate, w_gate_lora_a, w_gate_lora_b,
                 w_up, w_up_lora_a, w_up_lora_b,
                 w_down, w_down_lora_a, w_down_lora_b):
    """Fold LoRA, pad I, shard per core, and pre-tile DRAM layouts."""
    f32 = np.float32
    x1 = np.asarray(x1, f32)
    wg_eff = np.asarray(w_gate, f32) + np.asarray(w_gate_lora_a, f32) @ np.asarray(w_gate_lora_b, f32)
    wu_eff = np.asarray(w_up, f32) + np.asarray(w_up_lora_a, f32) @ np.asarray(w_up_lora_b, f32)
    wd_eff = np.asarray(w_down, f32) + np.asarray(w_down_lora_a, f32) @ np.asarray(w_down_lora_b, f32)

    wg_p = np.zeros((H, IPAD), f32); wg_p[:, :I] = wg_eff
    wu_p = np.zeros((H, IPAD), f32); wu_p[:, :I] = wu_eff
    wd_p = np.zeros((IPAD, H), f32); wd_p[:I, :] = wd_eff

    # x tile layout: x_tiled[b, p, k, t] = x2d[b*TB + t, k*128 + p]
    x2d = x1.reshape(NTOK, H)
    x_tiled = np.ascontiguousarray(
        x2d.reshape(NB, TB, K, 128).transpose(0, 3, 2, 1)
    ).astype(BF16).reshape(NB, 128, K * TB)

    in_maps = []
    for ci in range(NCORES):
        sl = slice(ci * IS, (ci + 1) * IS)
        # wg tile layout: [m, p, k, i] = wg_p[k*128+p, ci*IS + m*128 + i]
        wgc = np.ascontiguousarray(
            wg_p[:, sl].reshape(K, 128, C, 128).transpose(2, 1, 0, 3)
        ).astype(BF16).reshape(C, 128, K * 128)
        wuc = np.ascontiguousarray(
            wu_p[:, sl].reshape(K, 128, C, 128).transpose(2, 1, 0, 3)
        ).astype(BF16).reshape(C, 128, K * 128)
        # wd tile layout: [c, p, h] = wd_p[ci*IS + c*128 + p, h]
        wdc = wd_p[sl, :].reshape(C, 128, H).astype(BF16)
        in_maps.append({"x": x_tiled, "wg": wgc, "wu": wuc, "wd": wdc})
    return in_maps


def _emulate(in_maps):
    """Numpy emulation of the device math (bf16 operands, fp32 accum).
    Validates the host-side tilings and predicts the on-device accuracy."""
    f32 = np.float32
    acc = np.zeros((NTOK, H), f32)
    # reconstruct x2d (bf16-rounded) from the tiled layout
    xt = in_maps[0]["x"].reshape(NB, 128, K, TB)
    x2d = xt.transpose(0, 3, 2, 1).reshape(NTOK, H).astype(f32)
    for m in in_maps:
        wgc = m["wg"].reshape(C, 128, K, 128)
        wg2 = wgc.transpose(2, 1, 0, 3).reshape(H, IS).astype(f32)
        wuc = m["wu"].reshape(C, 128, K, 128)
        wu2 = wuc.transpose(2, 1, 0, 3).reshape(H, IS).astype(f32)
        wd2 = m["wd"].reshape(IS, H).astype(f32)
        y1 = x2d @ wg2
        y2 = x2d @ wu2
        r = np.maximum(y1, 0).astype(BF16).astype(f32)
        x3 = (r * y2).astype(BF16).astype(f32)
        acc += x3 @ wd2
    return acc.reshape(B, S, H)


def kernel(**inputs):
    global LAST_EXEC_TIME_NS, LAST_RESULTS
    in_maps = _prep_inputs(**inputs)

    if os.environ.get("KERNEL_EMULATE"):
        return _emulate(in_maps)

    from concourse.bass_utils import run_bass_kernel_spmd

    nc = _build_nc()
    res = run_bass_kernel_spmd(nc, in_maps, list(range(NCORES)), trace=TRACE)
    LAST_EXEC_TIME_NS = res.exec_time_ns
    LAST_RESULTS = res

    acc = np.zeros((NTOK, H), np.float32)
    for r in res.results:
        acc += r["out"]
    return acc.reshape(B, S, H)
